# revision 23
# baseline (speedup 1.0000x reference)
"""AGCN max/med fusion kernel for 8 TRN2 NeuronCores.

Data-parallel: 128 samples sharded 16 per core. Per sample:
  fre = channel-sum of fpam [1024,196]  -> argsort desc -> top16 + positions 89..104
  gather 32 feature columns, 1x1 conv (1024->256) + BN + ReLU per set (max/med)
  Lnorm graph matmul (16x16) per set; also emit rows/cols int32.

Device layout: fpam uploaded transposed per core as [16,196,1024] so
 - channel sum is a free-dim reduce (DVE/ACT)
 - node gather is a contiguous-row indirect DMA (4KB per node)
Pipelined in 8-sample halves: half-0 sort/gather/conv overlaps half-1 stream.
"""

import numpy as np

import concourse.bacc as bacc
import concourse.bass as bass
import concourse.mybir as mybir
import concourse.tile as tile
from concourse.bass_utils import run_bass_kernel_spmd

P = 128
NS = 16          # samples per core
NQ = 4           # samples per stream quarter
C = 1024
HW = 196
H = 14
O = 256
K = 16
NCORES = 8
NEG = -1.0e30
F32 = mybir.dt.float32
U32 = mybir.dt.uint32
I32 = mybir.dt.int32
BF16 = mybir.dt.bfloat16

_CACHE = {}


def _build_bass():
    nc = bacc.Bacc()

    fpamT = nc.declare_dram_parameter("fpamT", [NS, HW, C], F32, isOutput=False)
    w_pk = nc.declare_dram_parameter("w_pk", [2, P, 8 * O], F32, isOutput=False)
    bn_pk = nc.declare_dram_parameter("bn_pk", [P, 8], F32, isOutput=False)
    offs_d = nc.declare_dram_parameter("offs", [P, 4], U32, isOutput=False)
    ident_d = nc.declare_dram_parameter("ident", [P, P], F32, isOutput=False)

    out_max = nc.declare_dram_parameter("out_max", [NS, K * O], F32, isOutput=True)
    out_med = nc.declare_dram_parameter("out_med", [NS, K * O], F32, isOutput=True)
    out_rows = nc.declare_dram_parameter("out_rows", [NS, 2 * K], I32, isOutput=True)
    out_cols = nc.declare_dram_parameter("out_cols", [NS, 2 * K], I32, isOutput=True)

    X = mybir.AxisListType.X
    ALU = mybir.AluOpType
    ACTF = mybir.ActivationFunctionType

    with tile.TileContext(nc) as tc:
        with (
            tc.tile_pool(name="const", bufs=1) as constp,
            tc.tile_pool(name="stream", bufs=2) as streamp,
            tc.tile_pool(name="small", bufs=1) as smallp,
            tc.tile_pool(name="half", bufs=2) as halfp,
            tc.tile_pool(name="nodes", bufs=2) as nodesp,
            tc.tile_pool(name="psum", bufs=2, space="PSUM") as psump,
        ):
            # ---- constants (scalar ring; stream uses sync ring) ----
            ident = constp.tile([P, P], F32)
            nc.gpsimd.dma_start(out=ident[:], in_=ident_d[:])
            offs = constp.tile([P, 4], U32)
            nc.gpsimd.dma_start(out=offs[:], in_=offs_d[:])
            w_sb = {}
            for si, name in enumerate(("max", "med")):
                t = constp.tile([P, 8 * O], F32, tag=f"w_{name}", name=f"w_{name}")
                w_sb[name] = t
            bn_sb = constp.tile([P, 8], F32)
            nc.gpsimd.dma_start(out=bn_sb[:], in_=bn_pk[:])

            def w_lhsT(name, cb, ot):
                return w_sb[name][:, cb * O + ot * P:cb * O + (ot + 1) * P]

            def bn_ap(si, ot, which):
                col = si * 4 + ot * 2 + which
                return bn_sb[:, col:col + 1]

            # =========== per-half pipeline ===========
            for h in range(2):
                # ---- stream 8 samples; hw row (r*98+p) lives on partition p
                fre_s = halfp.tile([8, HW], F32, tag="fre_s", name="fre_s")
                HP = 98
                freq = halfp.tile([HP, 2, 8], F32, tag="freq", name="freq")
                for sl in range(8):
                    s = h * 8 + sl
                    tQ = streamp.tile([HP, 2, C], F32, tag="tQ", name="tQ",
                                      bufs=6)
                    # spread across the 3 DMA rings (gpsimd ring is slowest)
                    eng = (nc.sync, nc.scalar, nc.gpsimd, nc.sync,
                           nc.scalar, nc.sync, nc.scalar, nc.gpsimd)[sl]
                    eng.dma_start(
                        out=tQ[:], in_=fpamT[s].rearrange(
                            "(p r) c -> p r c", r=2))
                    # fre accumulate. Half 0: split DVE/ACT. Half 1: all ACT
                    # (DVE is busy with half-0 sort/tail at that point).
                    if h == 0 and sl % 2 == 0:
                        nc.vector.reduce_sum(
                            out=freq[:, :, sl:sl + 1], in_=tQ[:], axis=X)
                    else:
                        for r2 in range(2):
                            nc.scalar.activation(
                                out=tQ[:, r2], in_=tQ[:, r2], func=ACTF.Copy,
                                accum_out=freq[:, r2, sl:sl + 1])
                if h == 0:
                    # weight loads ride the rings behind the half-0 stream
                    nc.sync.dma_start(out=w_sb["max"][:], in_=w_pk[0])
                    nc.scalar.dma_start(out=w_sb["med"][:], in_=w_pk[1])
                # transpose each r2 block: [98, 8] -> [8, 98]
                for r2 in range(2):
                    psT = psump.tile([8, HP], F32, tag="frT", name="psT")
                    nc.tensor.transpose(out=psT[:], in_=freq[:, r2],
                                        identity=ident[0:HP, 0:HP])
                    nc.vector.tensor_copy(
                        out=fre_s[:, r2:HW:2], in_=psT[:])

                mx8 = halfp.tile([8, 8], F32, tag="mx8", name="mx8")
                idx_t = {}
                for r in range(14):
                    nc.vector.max(out=mx8[:], in_=fre_s[:])
                    if r in (0, 1, 11, 12, 13):
                        it = halfp.tile([8, 8], U32, tag=f"idx{r}", name=f"idx{r}")
                        nc.vector.max_index(out=it[:], in_max=mx8[:],
                                            in_values=fre_s[:])
                        idx_t[r] = it
                    if r < 13:
                        nc.vector.match_replace(out=fre_s[:], in_to_replace=mx8[:],
                                                in_values=fre_s[:], imm_value=NEG)

                scene = halfp.tile([8, 2 * K], U32, tag="scene", name="scene")
                nc.vector.tensor_copy(out=scene[:, 0:8], in_=idx_t[0][:])
                nc.vector.tensor_copy(out=scene[:, 8:16], in_=idx_t[1][:])
                nc.vector.tensor_copy(out=scene[:, 16:23], in_=idx_t[11][:, 1:8])
                nc.vector.tensor_copy(out=scene[:, 23:31], in_=idx_t[12][:])
                nc.vector.tensor_copy(out=scene[:, 31:32], in_=idx_t[13][:, 0:1])

                # ---- gather (start ASAP: only needs scene+offs) ----
                icol = halfp.tile([P, 2], U32, tag="icol", name="icol")
                for r in range(2):
                    nc.sync.dma_start(out=icol[:, r:r + 1],
                                      in_=scene[4 * r:4 * r + 4, :])
                gidx = halfp.tile([P, 2], U32, tag="gidx", name="gidx")
                nc.vector.tensor_tensor(out=gidx[:], in0=icol[:],
                                        in1=offs[:, 2 * h:2 * h + 2], op=ALU.add)

                fpam_flat = fpamT[:].flatten_outer_dims()
                nodesT = [nodesp.tile([P, 2 * P], F32, tag=f"nT{cb}",
                                      name=f"nT{cb}") for cb in range(8)]
                for r in range(2):
                    nd = nodesp.tile([P, C], F32, tag="nd", name="nd")
                    nc.gpsimd.indirect_dma_start(
                        out=nd[:], out_offset=None, in_=fpam_flat,
                        in_offset=bass.IndirectOffsetOnAxis(
                            ap=gidx[:, r:r + 1], axis=0),
                    )
                    for cb in range(8):
                        pst = psump.tile([P, P], F32, tag="tp", name="pst")
                        nc.tensor.transpose(out=pst[:],
                                            in_=nd[:, cb * P:(cb + 1) * P],
                                            identity=ident[:])
                        if (cb % 2 == 0) if h == 1 else (cb % 4 != 3):
                            nc.vector.tensor_copy(
                                out=nodesT[cb][:, r * P:(r + 1) * P], in_=pst[:])
                        else:
                            nc.scalar.activation(
                                out=nodesT[cb][:, r * P:(r + 1) * P], in_=pst[:],
                                func=ACTF.Copy)

                # ---- rows/cols (overlaps gather DMAs) ----
                scene_f = halfp.tile([8, 2 * K], F32, tag="scene_f", name="scene_f")
                nc.vector.tensor_copy(out=scene_f[:], in_=scene[:])
                rows_f = halfp.tile([8, 2 * K], F32, tag="rows_f", name="rows_f")
                cols_f = halfp.tile([8, 2 * K], F32, tag="cols_f", name="cols_f")
                tmp = halfp.tile([8, 2 * K], F32, tag="tmp", name="tmp")
                nc.vector.memset(rows_f[:], 0.0)
                for m in range(1, 14):
                    nc.vector.tensor_scalar(tmp[:], scene_f[:], float(14 * m),
                                            scalar2=None, op0=ALU.is_ge)
                    nc.vector.tensor_tensor(out=rows_f[:], in0=rows_f[:],
                                            in1=tmp[:], op=ALU.add)
                nc.vector.tensor_scalar(tmp[:], rows_f[:], float(H), scalar2=None,
                                        op0=ALU.mult)
                nc.vector.tensor_tensor(out=cols_f[:], in0=scene_f[:], in1=tmp[:],
                                        op=ALU.subtract)
                rows_i = halfp.tile([8, 2 * K], I32, tag="rows_i", name="rows_i")
                cols_i = halfp.tile([8, 2 * K], I32, tag="cols_i", name="cols_i")
                nc.vector.tensor_copy(out=rows_i[:], in_=rows_f[:])
                nc.vector.tensor_copy(out=cols_i[:], in_=cols_f[:])
                nc.sync.dma_start(out=out_rows[8 * h:8 * h + 8, :], in_=rows_i[:])
                nc.sync.dma_start(out=out_cols[8 * h:8 * h + 8, :], in_=cols_i[:])

                # ---- conv + BN + ReLU ----
                x_sb = {}
                for si, name in enumerate(("max", "med")):
                    for ot in range(2):
                        cv = psump.tile([P, P], F32, tag="conv", name="cv")
                        for cb in range(8):
                            rhs = nodesT[cb][:].rearrange("p (a b) -> p a b", b=32)
                            rhs = rhs[:, :, si * K:(si + 1) * K]
                            nc.tensor.matmul(
                                out=cv[:], lhsT=w_lhsT(name, cb, ot), rhs=rhs,
                                start=(cb == 0), stop=(cb == 7),
                            )
                        xt = halfp.tile([P, P], F32, tag=f"x_{name}_{ot}",
                                        name=f"x_{name}_{ot}")
                        nc.scalar.activation(out=xt[:], in_=cv[:], func=ACTF.Relu,
                                             scale=bn_ap(si, ot, 0),
                                             bias=bn_ap(si, ot, 1))
                        x_sb[name, ot] = xt

                # ---- Lnorm graphs ----
                for si, name in enumerate(("max", "med")):
                    off = si * K
                    dr = halfp.tile([8, K * K], F32, tag="dr", name="dr")
                    d2 = halfp.tile([8, K * K], F32, tag="d2", name="d2")
                    drv = dr[:].rearrange("p (a b) -> p a b", b=K)
                    nc.vector.tensor_tensor(
                        out=drv,
                        in0=rows_f[:, off:off + K][:, :, None].to_broadcast(
                            [8, K, K]),
                        in1=rows_f[:, off:off + K][:, None, :].to_broadcast(
                            [8, K, K]),
                        op=ALU.subtract)
                    nc.vector.tensor_tensor(out=d2[:], in0=dr[:], in1=dr[:],
                                            op=ALU.mult)
                    nc.vector.tensor_tensor(
                        out=drv,
                        in0=cols_f[:, off:off + K][:, :, None].to_broadcast(
                            [8, K, K]),
                        in1=cols_f[:, off:off + K][:, None, :].to_broadcast(
                            [8, K, K]),
                        op=ALU.subtract)
                    nc.vector.tensor_tensor(out=dr[:], in0=dr[:], in1=dr[:],
                                            op=ALU.mult)
                    nc.vector.tensor_tensor(out=d2[:], in0=d2[:], in1=dr[:],
                                            op=ALU.add)
                    nc.scalar.sqrt(out=d2[:], in_=d2[:])
                    nc.vector.tensor_scalar(d2[:], d2[:], 1.0, scalar2=None,
                                            op0=ALU.add)
                    A1 = halfp.tile([8, K * K], F32, tag="A1", name="A1")
                    nc.vector.reciprocal(out=A1[:], in_=d2[:])
                    dsum = halfp.tile([8, K], F32, tag="dsum", name="dsum")
                    nc.vector.reduce_sum(
                        out=dsum[:],
                        in_=A1[:].rearrange("p (a b) -> p a b", b=K), axis=X)
                    nc.scalar.sqrt(out=dsum[:], in_=dsum[:])
                    dinv = halfp.tile([8, K], F32, tag="dinv", name="dinv")
                    nc.vector.reciprocal(out=dinv[:], in_=dsum[:])

                    Bm = halfp.tile([P, P], F32, tag="Bblk", name="Bm")
                    nc.vector.memset(Bm[:], 0.0)
                    for t in range(8):
                        nc.sync.dma_start(
                            out=Bm[K * t:K * (t + 1), K * t:K * (t + 1)],
                            in_=A1[t:t + 1, :])
                    dcol = halfp.tile([P, 1], F32, tag="dcol", name="dcol")
                    nc.sync.dma_start(out=dcol[:], in_=dinv[:])

                    xT = halfp.tile([P, O], F32, tag="xT", name="xT")
                    for ot in range(2):
                        pst2 = psump.tile([P, P], F32, tag="tp", name="pst2")
                        nc.tensor.transpose(out=pst2[:], in_=x_sb[name, ot][:],
                                            identity=ident[:])
                        nc.vector.tensor_tensor(
                            out=xT[:, ot * P:(ot + 1) * P], in0=pst2[:],
                            in1=dcol[:].to_broadcast([P, P]), op=ALU.mult)
                    gout = psump.tile([P, O], F32, tag="gout", name="gout")
                    nc.tensor.matmul(out=gout[:], lhsT=Bm[:], rhs=xT[:],
                                     start=True, stop=True)
                    res = halfp.tile([P, O], F32, tag="res", name="res")
                    nc.vector.tensor_tensor(out=res[:], in0=gout[:],
                                            in1=dcol[:].to_broadcast([P, O]),
                                            op=ALU.mult)
                    outd = out_max if name == "max" else out_med
                    nc.sync.dma_start(out=outd[8 * h:8 * h + 8, :], in_=res[:])

    nc.compile()
    return nc


def _prep(inputs):
    fp = np.ascontiguousarray(inputs["fpam_output"], dtype=np.float32)
    N = fp.shape[0]
    fpamT = np.ascontiguousarray(
        fp.reshape(N, C, HW).transpose(0, 2, 1))          # [N, HW, C]

    def bn_fold(g, b, m, v):
        a = (g / np.sqrt(v + 1e-5)).astype(np.float32)
        bb = (b - m * a).astype(np.float32)
        return a, bb

    # packed weights: w_pk[set, p, cb*256+o] = w.T[cb*128+p, o]
    w_pk = np.empty((2, P, 8 * O), dtype=np.float32)
    for si, wname in enumerate(("w_max", "w_med")):
        wT = inputs[wname].T.astype(np.float32)           # [C, O]
        w_pk[si] = wT.reshape(8, P, O).transpose(1, 0, 2).reshape(P, 8 * O)

    # packed bn: bn_pk[p, si*4 + ot*2 + {a,b}]
    bn_pk = np.zeros((P, 8), dtype=np.float32)
    for si, nm in enumerate(("max", "med")):
        a, bb = bn_fold(inputs[f"g_{nm}"], inputs[f"b_{nm}"],
                        inputs[f"m_{nm}"], inputs[f"v_{nm}"])
        for ot in range(2):
            bn_pk[:, si * 4 + ot * 2 + 0] = a[ot * P:(ot + 1) * P]
            bn_pk[:, si * 4 + ot * 2 + 1] = bb[ot * P:(ot + 1) * P]

    j = np.arange(P)[:, None] + 128 * np.arange(4)[None, :]
    offs = ((j // (2 * K)) * HW).astype(np.uint32)        # [128, 4]
    ident = np.eye(P, dtype=np.float32)

    in_maps = []
    for c in range(NCORES):
        in_maps.append({
            "fpamT": fpamT[c * NS:(c + 1) * NS],
            "w_pk": w_pk, "bn_pk": bn_pk,
            "offs": offs, "ident": ident,
        })
    return in_maps


def kernel(**inputs):
    if "nc" not in _CACHE:
        _CACHE["nc"] = _build_bass()
    nc = _CACHE["nc"]
    in_maps = _prep(inputs)
    res = run_bass_kernel_spmd(nc, in_maps, core_ids=list(range(NCORES)))
    outs = res.results
    max_graph = np.concatenate([o["out_max"] for o in outs], axis=0)
    med_graph = np.concatenate([o["out_med"] for o in outs], axis=0)
    rows = np.concatenate([o["out_rows"] for o in outs], axis=0).astype(np.int32)
    cols = np.concatenate([o["out_cols"] for o in outs], axis=0).astype(np.int32)
    return (max_graph, med_graph, rows, cols)


# revision 26
# speedup vs baseline: 1.0056x; 1.0056x over previous
"""AGCN max/med fusion kernel for 8 TRN2 NeuronCores.

Data-parallel: 128 samples sharded 16 per core. Per sample:
  fre = channel-sum of fpam [1024,196]  -> argsort desc -> top16 + positions 89..104
  gather 32 feature columns, 1x1 conv (1024->256) + BN + ReLU per set (max/med)
  Lnorm graph matmul (16x16) per set; also emit rows/cols int32.

Device layout: fpam uploaded transposed per core as [16,196,1024] so
 - channel sum is a free-dim reduce (DVE/ACT)
 - node gather is a contiguous-row indirect DMA (4KB per node)
Pipelined in 8-sample halves: half-0 sort/gather/conv overlaps half-1 stream.
"""

import numpy as np

import concourse.bacc as bacc
import concourse.bass as bass
import concourse.mybir as mybir
import concourse.tile as tile
from concourse.bass_utils import run_bass_kernel_spmd

P = 128
NS = 16          # samples per core
NQ = 4           # samples per stream quarter
C = 1024
HW = 196
H = 14
O = 256
K = 16
NCORES = 8
NEG = -1.0e30
F32 = mybir.dt.float32
U32 = mybir.dt.uint32
I32 = mybir.dt.int32
BF16 = mybir.dt.bfloat16

_CACHE = {}


def _build_bass():
    nc = bacc.Bacc()

    fpamT = nc.declare_dram_parameter("fpamT", [NS, HW, C], F32, isOutput=False)
    w_pk = nc.declare_dram_parameter("w_pk", [2, P, 8 * O], F32, isOutput=False)
    bn_pk = nc.declare_dram_parameter("bn_pk", [P, 8], F32, isOutput=False)
    offs_d = nc.declare_dram_parameter("offs", [P, 4], U32, isOutput=False)
    ident_d = nc.declare_dram_parameter("ident", [P, P], F32, isOutput=False)

    out_max = nc.declare_dram_parameter("out_max", [NS, K * O], F32, isOutput=True)
    out_med = nc.declare_dram_parameter("out_med", [NS, K * O], F32, isOutput=True)
    out_rows = nc.declare_dram_parameter("out_rows", [NS, 2 * K], I32, isOutput=True)
    out_cols = nc.declare_dram_parameter("out_cols", [NS, 2 * K], I32, isOutput=True)

    X = mybir.AxisListType.X
    ALU = mybir.AluOpType
    ACTF = mybir.ActivationFunctionType

    with tile.TileContext(nc) as tc:
        with (
            tc.tile_pool(name="const", bufs=1) as constp,
            tc.tile_pool(name="stream", bufs=2) as streamp,
            tc.tile_pool(name="small", bufs=1) as smallp,
            tc.tile_pool(name="half", bufs=2) as halfp,
            tc.tile_pool(name="nodes", bufs=2) as nodesp,
            tc.tile_pool(name="psum", bufs=2, space="PSUM") as psump,
        ):
            # ---- constants (scalar ring; stream uses sync ring) ----
            ident = constp.tile([P, P], F32)
            nc.gpsimd.dma_start(out=ident[:], in_=ident_d[:])
            offs = constp.tile([P, 4], U32)
            nc.gpsimd.dma_start(out=offs[:], in_=offs_d[:])
            w_sb = {}
            for si, name in enumerate(("max", "med")):
                t = constp.tile([P, 8 * O], F32, tag=f"w_{name}", name=f"w_{name}")
                w_sb[name] = t
            bn_sb = constp.tile([P, 8], F32)
            nc.gpsimd.dma_start(out=bn_sb[:], in_=bn_pk[:])

            def w_lhsT(name, cb, ot):
                return w_sb[name][:, cb * O + ot * P:cb * O + (ot + 1) * P]

            def bn_ap(si, ot, which):
                col = si * 4 + ot * 2 + which
                return bn_sb[:, col:col + 1]

            # =========== per-half pipeline ===========
            fpam_rows = fpamT[:].flatten_outer_dims()     # [3136, 1024]
            for h in range(2):
                # ---- stream: 2-sample groups of 392 rows = [128,3]+8 rem.
                # All 128 partitions per big DMA -> full DMA-port bandwidth.
                fre_s = halfp.tile([8, HW], F32, tag="fre_s", name="fre_s")
                base = h * 8 * HW
                # packed remnant rows (4 groups x 8 rows) via gpsimd ring
                tR = streamp.tile([32, C], F32, tag="tR", name="tR", bufs=2)
                rem_src = fpam_rows[base:base + 4 * 392].rearrange(
                    "(g r) c -> g r c", r=392)[:, 384:392, :]
                nc.gpsimd.dma_start(out=tR[:], in_=rem_src)
                frR = halfp.tile([32, 1], F32, tag="frR", name="frR")
                if h == 0:
                    nc.vector.reduce_sum(out=frR[:], in_=tR[:], axis=X)
                else:
                    nc.scalar.activation(out=tR[:], in_=tR[:], func=ACTF.Copy,
                                         accum_out=frR[:])
                psR = psump.tile([1, 32], F32, tag="frT", name="psR")
                nc.tensor.transpose(out=psR[:], in_=frR[:],
                                    identity=ident[0:32, 0:32])
                stgR = halfp.tile([1, 32], F32, tag="stgR", name="stgR")
                nc.vector.tensor_copy(out=stgR[:], in_=psR[:])
                for t in range(4):
                    nc.gpsimd.dma_start(
                        out=fre_s[2 * t + 1:2 * t + 2, 188:196],
                        in_=stgR[0:1, 8 * t:8 * t + 8])
                for j in range(4):
                    row0 = base + j * 392
                    tG = streamp.tile([P, 3, C], F32, tag="tG", name="tG",
                                      bufs=4)
                    eng = (nc.sync, nc.scalar)[j % 2]
                    eng.dma_start(out=tG[:],
                                  in_=fpam_rows[row0:row0 + 384].rearrange(
                                      "(i p) c -> p i c", p=P))
                    frG = halfp.tile([P, 3], F32, tag=f"frG{j}", name="frG")
                    if h == 0 and j % 2 == 0:
                        nc.vector.reduce_sum(out=frG[:], in_=tG[:], axis=X)
                    else:
                        for i in range(3):
                            nc.scalar.activation(
                                out=tG[:, i], in_=tG[:, i], func=ACTF.Copy,
                                accum_out=frG[:, i:i + 1])
                    psG = psump.tile([3, P], F32, tag="frT", name="psG")
                    nc.tensor.transpose(out=psG[:], in_=frG[:], identity=ident[:])
                    stg3 = halfp.tile([3, P], F32, tag=f"stg{j}", name="stg3")
                    nc.vector.tensor_copy(out=stg3[:], in_=psG[:])
                    # local row L = 128*i + p; sample 2j + L//196, hw = L%196
                    s0 = 2 * j
                    nc.gpsimd.dma_start(out=fre_s[s0:s0 + 1, 0:128],
                                        in_=stg3[0:1, :])
                    nc.gpsimd.dma_start(out=fre_s[s0:s0 + 1, 128:196],
                                        in_=stg3[1:2, 0:68])
                    nc.gpsimd.dma_start(out=fre_s[s0 + 1:s0 + 2, 0:60],
                                        in_=stg3[1:2, 68:128])
                    nc.gpsimd.dma_start(out=fre_s[s0 + 1:s0 + 2, 60:188],
                                        in_=stg3[2:3, :])
                if h == 0:
                    # weight loads ride the rings behind the half-0 stream
                    nc.sync.dma_start(out=w_sb["max"][:], in_=w_pk[0])
                    nc.scalar.dma_start(out=w_sb["med"][:], in_=w_pk[1])

                mx8 = halfp.tile([8, 8], F32, tag="mx8", name="mx8")
                idx_t = {}
                for r in range(14):
                    nc.vector.max(out=mx8[:], in_=fre_s[:])
                    if r in (0, 1, 11, 12, 13):
                        it = halfp.tile([8, 8], U32, tag=f"idx{r}", name=f"idx{r}")
                        nc.vector.max_index(out=it[:], in_max=mx8[:],
                                            in_values=fre_s[:])
                        idx_t[r] = it
                    if r < 13:
                        nc.vector.match_replace(out=fre_s[:], in_to_replace=mx8[:],
                                                in_values=fre_s[:], imm_value=NEG)

                scene = halfp.tile([8, 2 * K], U32, tag="scene", name="scene")
                nc.vector.tensor_copy(out=scene[:, 0:8], in_=idx_t[0][:])
                nc.vector.tensor_copy(out=scene[:, 8:16], in_=idx_t[1][:])
                nc.vector.tensor_copy(out=scene[:, 16:23], in_=idx_t[11][:, 1:8])
                nc.vector.tensor_copy(out=scene[:, 23:31], in_=idx_t[12][:])
                nc.vector.tensor_copy(out=scene[:, 31:32], in_=idx_t[13][:, 0:1])

                # ---- gather (start ASAP: only needs scene+offs) ----
                icol = halfp.tile([P, 2], U32, tag="icol", name="icol")
                for r in range(2):
                    nc.gpsimd.dma_start(out=icol[:, r:r + 1],
                                        in_=scene[4 * r:4 * r + 4, :])
                gidx = halfp.tile([P, 2], U32, tag="gidx", name="gidx")
                nc.vector.tensor_tensor(out=gidx[:], in0=icol[:],
                                        in1=offs[:, 2 * h:2 * h + 2], op=ALU.add)

                fpam_flat = fpamT[:].flatten_outer_dims()
                nodesT = [nodesp.tile([P, 2 * P], F32, tag=f"nT{cb}",
                                      name=f"nT{cb}") for cb in range(8)]
                for r in range(2):
                    nd = nodesp.tile([P, C], F32, tag="nd", name="nd")
                    nc.gpsimd.indirect_dma_start(
                        out=nd[:], out_offset=None, in_=fpam_flat,
                        in_offset=bass.IndirectOffsetOnAxis(
                            ap=gidx[:, r:r + 1], axis=0),
                    )
                    for cb in range(8):
                        pst = psump.tile([P, P], F32, tag="tp", name="pst")
                        nc.tensor.transpose(out=pst[:],
                                            in_=nd[:, cb * P:(cb + 1) * P],
                                            identity=ident[:])
                        if (cb % 2 == 0) if h == 1 else (cb % 4 != 3):
                            nc.vector.tensor_copy(
                                out=nodesT[cb][:, r * P:(r + 1) * P], in_=pst[:])
                        else:
                            nc.scalar.activation(
                                out=nodesT[cb][:, r * P:(r + 1) * P], in_=pst[:],
                                func=ACTF.Copy)

                # ---- rows/cols (overlaps gather DMAs) ----
                scene_f = halfp.tile([8, 2 * K], F32, tag="scene_f", name="scene_f")
                nc.vector.tensor_copy(out=scene_f[:], in_=scene[:])
                rows_f = halfp.tile([8, 2 * K], F32, tag="rows_f", name="rows_f")
                cols_f = halfp.tile([8, 2 * K], F32, tag="cols_f", name="cols_f")
                tmp = halfp.tile([8, 2 * K], F32, tag="tmp", name="tmp")
                nc.vector.memset(rows_f[:], 0.0)
                for m in range(1, 14):
                    nc.vector.tensor_scalar(tmp[:], scene_f[:], float(14 * m),
                                            scalar2=None, op0=ALU.is_ge)
                    nc.vector.tensor_tensor(out=rows_f[:], in0=rows_f[:],
                                            in1=tmp[:], op=ALU.add)
                nc.vector.tensor_scalar(tmp[:], rows_f[:], float(H), scalar2=None,
                                        op0=ALU.mult)
                nc.vector.tensor_tensor(out=cols_f[:], in0=scene_f[:], in1=tmp[:],
                                        op=ALU.subtract)
                rows_i = halfp.tile([8, 2 * K], I32, tag="rows_i", name="rows_i")
                cols_i = halfp.tile([8, 2 * K], I32, tag="cols_i", name="cols_i")
                nc.vector.tensor_copy(out=rows_i[:], in_=rows_f[:])
                nc.vector.tensor_copy(out=cols_i[:], in_=cols_f[:])
                nc.gpsimd.dma_start(out=out_rows[8 * h:8 * h + 8, :], in_=rows_i[:])
                nc.gpsimd.dma_start(out=out_cols[8 * h:8 * h + 8, :], in_=cols_i[:])

                # ---- conv + BN + ReLU ----
                x_sb = {}
                for si, name in enumerate(("max", "med")):
                    for ot in range(2):
                        cv = psump.tile([P, P], F32, tag="conv", name="cv")
                        for cb in range(8):
                            rhs = nodesT[cb][:].rearrange("p (a b) -> p a b", b=32)
                            rhs = rhs[:, :, si * K:(si + 1) * K]
                            nc.tensor.matmul(
                                out=cv[:], lhsT=w_lhsT(name, cb, ot), rhs=rhs,
                                start=(cb == 0), stop=(cb == 7),
                            )
                        xt = halfp.tile([P, P], F32, tag=f"x_{name}_{ot}",
                                        name=f"x_{name}_{ot}")
                        nc.scalar.activation(out=xt[:], in_=cv[:], func=ACTF.Relu,
                                             scale=bn_ap(si, ot, 0),
                                             bias=bn_ap(si, ot, 1))
                        x_sb[name, ot] = xt

                # ---- Lnorm graphs ----
                for si, name in enumerate(("max", "med")):
                    off = si * K
                    dr = halfp.tile([8, K * K], F32, tag="dr", name="dr")
                    d2 = halfp.tile([8, K * K], F32, tag="d2", name="d2")
                    drv = dr[:].rearrange("p (a b) -> p a b", b=K)
                    nc.vector.tensor_tensor(
                        out=drv,
                        in0=rows_f[:, off:off + K][:, :, None].to_broadcast(
                            [8, K, K]),
                        in1=rows_f[:, off:off + K][:, None, :].to_broadcast(
                            [8, K, K]),
                        op=ALU.subtract)
                    nc.vector.tensor_tensor(out=d2[:], in0=dr[:], in1=dr[:],
                                            op=ALU.mult)
                    nc.vector.tensor_tensor(
                        out=drv,
                        in0=cols_f[:, off:off + K][:, :, None].to_broadcast(
                            [8, K, K]),
                        in1=cols_f[:, off:off + K][:, None, :].to_broadcast(
                            [8, K, K]),
                        op=ALU.subtract)
                    nc.vector.tensor_tensor(out=dr[:], in0=dr[:], in1=dr[:],
                                            op=ALU.mult)
                    nc.vector.tensor_tensor(out=d2[:], in0=d2[:], in1=dr[:],
                                            op=ALU.add)
                    nc.scalar.sqrt(out=d2[:], in_=d2[:])
                    nc.vector.tensor_scalar(d2[:], d2[:], 1.0, scalar2=None,
                                            op0=ALU.add)
                    A1 = halfp.tile([8, K * K], F32, tag="A1", name="A1")
                    nc.vector.reciprocal(out=A1[:], in_=d2[:])
                    dsum = halfp.tile([8, K], F32, tag="dsum", name="dsum")
                    nc.vector.reduce_sum(
                        out=dsum[:],
                        in_=A1[:].rearrange("p (a b) -> p a b", b=K), axis=X)
                    nc.scalar.sqrt(out=dsum[:], in_=dsum[:])
                    dinv = halfp.tile([8, K], F32, tag="dinv", name="dinv")
                    nc.vector.reciprocal(out=dinv[:], in_=dsum[:])

                    Bm = halfp.tile([P, P], F32, tag="Bblk", name="Bm")
                    nc.vector.memset(Bm[:], 0.0)
                    for t in range(8):
                        nc.gpsimd.dma_start(
                            out=Bm[K * t:K * (t + 1), K * t:K * (t + 1)],
                            in_=A1[t:t + 1, :])
                    dcol = halfp.tile([P, 1], F32, tag="dcol", name="dcol")
                    nc.gpsimd.dma_start(out=dcol[:], in_=dinv[:])

                    xT = halfp.tile([P, O], F32, tag="xT", name="xT")
                    for ot in range(2):
                        pst2 = psump.tile([P, P], F32, tag="tp", name="pst2")
                        nc.tensor.transpose(out=pst2[:], in_=x_sb[name, ot][:],
                                            identity=ident[:])
                        nc.vector.tensor_tensor(
                            out=xT[:, ot * P:(ot + 1) * P], in0=pst2[:],
                            in1=dcol[:].to_broadcast([P, P]), op=ALU.mult)
                    gout = psump.tile([P, O], F32, tag="gout", name="gout")
                    nc.tensor.matmul(out=gout[:], lhsT=Bm[:], rhs=xT[:],
                                     start=True, stop=True)
                    res = halfp.tile([P, O], F32, tag="res", name="res")
                    nc.vector.tensor_tensor(out=res[:], in0=gout[:],
                                            in1=dcol[:].to_broadcast([P, O]),
                                            op=ALU.mult)
                    outd = out_max if name == "max" else out_med
                    nc.sync.dma_start(out=outd[8 * h:8 * h + 8, :], in_=res[:])

    nc.compile()
    return nc


def _prep(inputs):
    fp = np.ascontiguousarray(inputs["fpam_output"], dtype=np.float32)
    N = fp.shape[0]
    fpamT = np.ascontiguousarray(
        fp.reshape(N, C, HW).transpose(0, 2, 1))          # [N, HW, C]

    def bn_fold(g, b, m, v):
        a = (g / np.sqrt(v + 1e-5)).astype(np.float32)
        bb = (b - m * a).astype(np.float32)
        return a, bb

    # packed weights: w_pk[set, p, cb*256+o] = w.T[cb*128+p, o]
    w_pk = np.empty((2, P, 8 * O), dtype=np.float32)
    for si, wname in enumerate(("w_max", "w_med")):
        wT = inputs[wname].T.astype(np.float32)           # [C, O]
        w_pk[si] = wT.reshape(8, P, O).transpose(1, 0, 2).reshape(P, 8 * O)

    # packed bn: bn_pk[p, si*4 + ot*2 + {a,b}]
    bn_pk = np.zeros((P, 8), dtype=np.float32)
    for si, nm in enumerate(("max", "med")):
        a, bb = bn_fold(inputs[f"g_{nm}"], inputs[f"b_{nm}"],
                        inputs[f"m_{nm}"], inputs[f"v_{nm}"])
        for ot in range(2):
            bn_pk[:, si * 4 + ot * 2 + 0] = a[ot * P:(ot + 1) * P]
            bn_pk[:, si * 4 + ot * 2 + 1] = bb[ot * P:(ot + 1) * P]

    j = np.arange(P)[:, None] + 128 * np.arange(4)[None, :]
    offs = ((j // (2 * K)) * HW).astype(np.uint32)        # [128, 4]
    ident = np.eye(P, dtype=np.float32)

    in_maps = []
    for c in range(NCORES):
        in_maps.append({
            "fpamT": fpamT[c * NS:(c + 1) * NS],
            "w_pk": w_pk, "bn_pk": bn_pk,
            "offs": offs, "ident": ident,
        })
    return in_maps


def kernel(**inputs):
    if "nc" not in _CACHE:
        _CACHE["nc"] = _build_bass()
    nc = _CACHE["nc"]
    in_maps = _prep(inputs)
    res = run_bass_kernel_spmd(nc, in_maps, core_ids=list(range(NCORES)))
    outs = res.results
    max_graph = np.concatenate([o["out_max"] for o in outs], axis=0)
    med_graph = np.concatenate([o["out_med"] for o in outs], axis=0)
    rows = np.concatenate([o["out_rows"] for o in outs], axis=0).astype(np.int32)
    cols = np.concatenate([o["out_cols"] for o in outs], axis=0).astype(np.int32)
    return (max_graph, med_graph, rows, cols)


# revision 27
# speedup vs baseline: 1.1296x; 1.1234x over previous
"""AGCN max/med fusion kernel for 8 TRN2 NeuronCores.

Data-parallel: 128 samples sharded 16 per core. Per sample:
  fre = channel-sum of fpam [1024,196]  -> argsort desc -> top16 + positions 89..104
  gather 32 feature columns, 1x1 conv (1024->256) + BN + ReLU per set (max/med)
  Lnorm graph matmul (16x16) per set; also emit rows/cols int32.

Layout/bandwidth tricks:
 - fpam uploaded transposed per core as [16,196,1024]; streamed as 2-sample
   groups of [128,3,1024] (128 partitions -> full DMA-port bandwidth) plus a
   packed remnant stream, split across the sync+scalar HWDGE rings.
 - channel sum = free-dim reduce, split DVE/ACT; fre assembled via tiny DMAs.
 - sort = 14 rounds of max8/match_replace on [16,196], all samples at once.
 - node gather = contiguous-row indirect DMA (4KB per node row).
 - conv via PE with PE-transposed nodesT; BN folded into ACT scale/bias+ReLU.
 - Lnorm graph matmul via block-diagonal symmetric affinity with dinv folded
   into row scalings.
"""

import numpy as np

import concourse.bacc as bacc
import concourse.bass as bass
import concourse.mybir as mybir
import concourse.tile as tile
from concourse.bass_utils import run_bass_kernel_spmd

P = 128
NS = 16          # samples per core
C = 1024
HW = 196
H = 14
O = 256
K = 16
NCORES = 8
NEG = -1.0e30
F32 = mybir.dt.float32
U32 = mybir.dt.uint32
I32 = mybir.dt.int32

_CACHE = {}


def _build_bass():
    nc = bacc.Bacc()

    fpamT = nc.declare_dram_parameter("fpamT", [NS, HW, C], F32, isOutput=False)
    w_pk = nc.declare_dram_parameter("w_pk", [2, P, 8 * O], F32, isOutput=False)
    bn_pk = nc.declare_dram_parameter("bn_pk", [P, 8], F32, isOutput=False)
    offs_d = nc.declare_dram_parameter("offs", [P, 4], U32, isOutput=False)
    ident_d = nc.declare_dram_parameter("ident", [P, P], F32, isOutput=False)

    out_max = nc.declare_dram_parameter("out_max", [NS, K * O], F32, isOutput=True)
    out_med = nc.declare_dram_parameter("out_med", [NS, K * O], F32, isOutput=True)
    out_rows = nc.declare_dram_parameter("out_rows", [NS, 2 * K], I32, isOutput=True)
    out_cols = nc.declare_dram_parameter("out_cols", [NS, 2 * K], I32, isOutput=True)

    X = mybir.AxisListType.X
    ALU = mybir.AluOpType
    ACTF = mybir.ActivationFunctionType

    with tile.TileContext(nc) as tc:
        with (
            tc.tile_pool(name="const", bufs=1) as constp,
            tc.tile_pool(name="stream", bufs=2) as streamp,
            tc.tile_pool(name="small", bufs=1) as smallp,
            tc.tile_pool(name="nodes", bufs=1) as nodesp,
            tc.tile_pool(name="psum", bufs=2, space="PSUM") as psump,
        ):
            # ---- constants on the gpsimd ring (stream owns sync/scalar) ----
            ident = constp.tile([P, P], F32)
            nc.gpsimd.dma_start(out=ident[:], in_=ident_d[:])
            offs = constp.tile([P, 4], U32)
            nc.gpsimd.dma_start(out=offs[:], in_=offs_d[:])
            bn_sb = constp.tile([P, 8], F32)
            nc.gpsimd.dma_start(out=bn_sb[:], in_=bn_pk[:])
            w_sb = {}
            for si, name in enumerate(("max", "med")):
                t = constp.tile([P, 8 * O], F32, tag=f"w_{name}", name=f"w_{name}")
                w_sb[name] = t

            def w_lhsT(name, cb, ot):
                return w_sb[name][:, cb * O + ot * P:cb * O + (ot + 1) * P]

            def bn_ap(si, ot, which):
                col = si * 4 + ot * 2 + which
                return bn_sb[:, col:col + 1]

            # ======== stream all 16 samples; assemble fre_s [16, 196] ========
            fpam_rows = fpamT[:].flatten_outer_dims()     # [3136, 1024]
            fre_s = smallp.tile([NS, HW], F32)
            for h in range(2):
                base = h * 8 * HW
                # packed remnant rows (4 groups x 8 rows), gpsimd ring
                tR = streamp.tile([32, C], F32, tag="tR", name="tR", bufs=2)
                rem_src = fpam_rows[base:base + 4 * 392].rearrange(
                    "(g r) c -> g r c", r=392)[:, 384:392, :]
                nc.gpsimd.dma_start(out=tR[:], in_=rem_src)
                frR = smallp.tile([32, 1], F32, tag=f"frR{h}", name="frR")
                if h == 0:
                    nc.vector.reduce_sum(out=frR[:], in_=tR[:], axis=X)
                else:
                    nc.scalar.activation(out=tR[:], in_=tR[:], func=ACTF.Copy,
                                         accum_out=frR[:])
                psR = psump.tile([1, 32], F32, tag="frT", name="psR")
                nc.tensor.transpose(out=psR[:], in_=frR[:],
                                    identity=ident[0:32, 0:32])
                stgR = smallp.tile([1, 32], F32, tag=f"stgR{h}", name="stgR")
                nc.vector.tensor_copy(out=stgR[:], in_=psR[:])
                seg_eng = nc.gpsimd if h == 0 else nc.sync
                for t in range(4):
                    seg_eng.dma_start(
                        out=fre_s[8 * h + 2 * t + 1:8 * h + 2 * t + 2, 188:196],
                        in_=stgR[0:1, 8 * t:8 * t + 8])
                for j in range(4):
                    row0 = base + j * 392
                    tG = streamp.tile([P, 3, C], F32, tag="tG", name="tG",
                                      bufs=4)
                    eng = (nc.sync, nc.scalar)[j % 2]
                    eng.dma_start(out=tG[:],
                                  in_=fpam_rows[row0:row0 + 384].rearrange(
                                      "(i p) c -> p i c", p=P))
                    frG = smallp.tile([P, 3], F32, tag=f"frG{h}{j}", name="frG")
                    if j % 2 == 0:
                        nc.vector.reduce_sum(out=frG[:], in_=tG[:], axis=X)
                    else:
                        for i in range(3):
                            nc.scalar.activation(
                                out=tG[:, i], in_=tG[:, i], func=ACTF.Copy,
                                accum_out=frG[:, i:i + 1])
                    psG = psump.tile([3, P], F32, tag="frT", name="psG")
                    nc.tensor.transpose(out=psG[:], in_=frG[:], identity=ident[:])
                    stg3 = smallp.tile([3, P], F32, tag=f"stg{h}{j}", name="stg3")
                    nc.vector.tensor_copy(out=stg3[:], in_=psG[:])
                    # local row L = 128*i + p -> sample 8h+2j + L//196, hw=L%196
                    s0 = 8 * h + 2 * j
                    seg_eng.dma_start(out=fre_s[s0:s0 + 1, 0:128],
                                      in_=stg3[0:1, :])
                    seg_eng.dma_start(out=fre_s[s0:s0 + 1, 128:196],
                                      in_=stg3[1:2, 0:68])
                    seg_eng.dma_start(out=fre_s[s0 + 1:s0 + 2, 0:60],
                                      in_=stg3[1:2, 68:128])
                    seg_eng.dma_start(out=fre_s[s0 + 1:s0 + 2, 60:188],
                                      in_=stg3[2:3, :])
                if h == 0:
                    # weight loads ride the rings behind the half-0 stream
                    nc.sync.dma_start(out=w_sb["max"][:], in_=w_pk[0])
                    nc.scalar.dma_start(out=w_sb["med"][:], in_=w_pk[1])

            # ======== sort: top-8 x 14 rounds over all 16 samples ========
            mx8 = smallp.tile([NS, 8], F32)
            idx_t = {}
            for r in range(14):
                nc.vector.max(out=mx8[:], in_=fre_s[:])
                if r in (0, 1, 11, 12, 13):
                    it = smallp.tile([NS, 8], U32, tag=f"idx{r}", name=f"idx{r}")
                    nc.vector.max_index(out=it[:], in_max=mx8[:],
                                        in_values=fre_s[:])
                    idx_t[r] = it
                if r < 13:
                    nc.vector.match_replace(out=fre_s[:], in_to_replace=mx8[:],
                                            in_values=fre_s[:], imm_value=NEG)

            scene = smallp.tile([NS, 2 * K], U32)
            nc.vector.tensor_copy(out=scene[:, 0:8], in_=idx_t[0][:])
            nc.vector.tensor_copy(out=scene[:, 8:16], in_=idx_t[1][:])
            nc.vector.tensor_copy(out=scene[:, 16:23], in_=idx_t[11][:, 1:8])
            nc.vector.tensor_copy(out=scene[:, 23:31], in_=idx_t[12][:])
            nc.vector.tensor_copy(out=scene[:, 31:32], in_=idx_t[13][:, 0:1])

            # ======== gather: 4x indirect DMA of 128 rows each ========
            icol = smallp.tile([P, 4], U32)
            for r in range(4):
                nc.sync.dma_start(out=icol[:, r:r + 1],
                                  in_=scene[4 * r:4 * r + 4, :])
            gidx = smallp.tile([P, 4], U32)
            nc.vector.tensor_tensor(out=gidx[:], in0=icol[:], in1=offs[:],
                                    op=ALU.add)
            nodesT = [nodesp.tile([P, 4 * P], F32, tag=f"nT{cb}", name=f"nT{cb}")
                      for cb in range(8)]
            for r in range(4):
                nd = nodesp.tile([P, C], F32, tag="nd", name="nd", bufs=2)
                nc.gpsimd.indirect_dma_start(
                    out=nd[:], out_offset=None, in_=fpam_rows,
                    in_offset=bass.IndirectOffsetOnAxis(ap=gidx[:, r:r + 1],
                                                        axis=0))
                for cb in range(8):
                    pst = psump.tile([P, P], F32, tag="tp", name="pst")
                    nc.tensor.transpose(out=pst[:], in_=nd[:, cb * P:(cb + 1) * P],
                                        identity=ident[:])
                    nc.vector.tensor_copy(out=nodesT[cb][:, r * P:(r + 1) * P],
                                          in_=pst[:])

            # ======== rows/cols (parallel to gather) ========
            scene_f = smallp.tile([NS, 2 * K], F32)
            nc.vector.tensor_copy(out=scene_f[:], in_=scene[:])
            rows_f = smallp.tile([NS, 2 * K], F32)
            cols_f = smallp.tile([NS, 2 * K], F32)
            tmp = smallp.tile([NS, 2 * K], F32)
            nc.vector.memset(rows_f[:], 0.0)
            for m in range(1, 14):
                nc.vector.tensor_scalar(tmp[:], scene_f[:], float(14 * m),
                                        scalar2=None, op0=ALU.is_ge)
                nc.vector.tensor_tensor(out=rows_f[:], in0=rows_f[:], in1=tmp[:],
                                        op=ALU.add)
            nc.vector.tensor_scalar(tmp[:], rows_f[:], float(H), scalar2=None,
                                    op0=ALU.mult)
            nc.vector.tensor_tensor(out=cols_f[:], in0=scene_f[:], in1=tmp[:],
                                    op=ALU.subtract)
            rows_i = smallp.tile([NS, 2 * K], I32)
            cols_i = smallp.tile([NS, 2 * K], I32)
            nc.vector.tensor_copy(out=rows_i[:], in_=rows_f[:])
            nc.vector.tensor_copy(out=cols_i[:], in_=cols_f[:])
            nc.scalar.dma_start(out=out_rows[:], in_=rows_i[:])
            nc.scalar.dma_start(out=out_cols[:], in_=cols_i[:])

            # ======== conv + BN + ReLU ========
            x_sb = {}
            for si, name in enumerate(("max", "med")):
                for ot in range(2):
                    cv = psump.tile([P, O], F32, tag="conv", name="cv")
                    for cb in range(8):
                        rhs = nodesT[cb][:].rearrange("p (a b) -> p a b", b=32)
                        rhs = rhs[:, :, si * K:(si + 1) * K]
                        nc.tensor.matmul(out=cv[:], lhsT=w_lhsT(name, cb, ot),
                                         rhs=rhs, start=(cb == 0), stop=(cb == 7))
                    xt = smallp.tile([P, O], F32, tag=f"x_{name}_{ot}",
                                     name=f"x_{name}_{ot}")
                    nc.scalar.activation(out=xt[:], in_=cv[:], func=ACTF.Relu,
                                         scale=bn_ap(si, ot, 0),
                                         bias=bn_ap(si, ot, 1))
                    x_sb[name, ot] = xt

            # ======== Lnorm graphs ========
            for si, name in enumerate(("max", "med")):
                off = si * K
                dr = smallp.tile([NS, K * K], F32, tag="dr", name="dr")
                d2 = smallp.tile([NS, K * K], F32, tag="d2", name="d2")
                drv = dr[:].rearrange("p (a b) -> p a b", b=K)
                nc.vector.tensor_tensor(
                    out=drv,
                    in0=rows_f[:, off:off + K][:, :, None].to_broadcast(
                        [NS, K, K]),
                    in1=rows_f[:, off:off + K][:, None, :].to_broadcast(
                        [NS, K, K]),
                    op=ALU.subtract)
                nc.vector.tensor_tensor(out=d2[:], in0=dr[:], in1=dr[:],
                                        op=ALU.mult)
                nc.vector.tensor_tensor(
                    out=drv,
                    in0=cols_f[:, off:off + K][:, :, None].to_broadcast(
                        [NS, K, K]),
                    in1=cols_f[:, off:off + K][:, None, :].to_broadcast(
                        [NS, K, K]),
                    op=ALU.subtract)
                nc.vector.tensor_tensor(out=dr[:], in0=dr[:], in1=dr[:],
                                        op=ALU.mult)
                nc.vector.tensor_tensor(out=d2[:], in0=d2[:], in1=dr[:],
                                        op=ALU.add)
                nc.scalar.sqrt(out=d2[:], in_=d2[:])
                nc.vector.tensor_scalar(d2[:], d2[:], 1.0, scalar2=None,
                                        op0=ALU.add)
                A1 = smallp.tile([NS, K * K], F32, tag=f"A1{si}", name="A1")
                nc.vector.reciprocal_approx_fast(out=A1[:], in_=d2[:])
                dsum = smallp.tile([NS, K], F32, tag="dsum", name="dsum")
                nc.vector.reduce_sum(
                    out=dsum[:], in_=A1[:].rearrange("p (a b) -> p a b", b=K),
                    axis=X)
                nc.scalar.sqrt(out=dsum[:], in_=dsum[:])
                dinv = smallp.tile([NS, K], F32, tag=f"dinv{si}", name="dinv")
                nc.vector.reciprocal_approx_fast(out=dinv[:], in_=dsum[:])

                outd = out_max if name == "max" else out_med
                for g in range(2):
                    Bm = smallp.tile([P, P], F32, tag=f"Bb_{name}_{g}", name="Bm")
                    nc.vector.memset(Bm[:], 0.0)
                    for t in range(8):
                        nc.scalar.dma_start(
                            out=Bm[K * t:K * (t + 1), K * t:K * (t + 1)],
                            in_=A1[8 * g + t:8 * g + t + 1, :])
                    dcol = smallp.tile([P, 1], F32, tag=f"dc_{name}_{g}",
                                       name="dcol")
                    nc.scalar.dma_start(out=dcol[:],
                                        in_=dinv[8 * g:8 * (g + 1), :])

                    xT = smallp.tile([P, O], F32, tag=f"xT_{name}_{g}", name="xT")
                    for ot in range(2):
                        pst2 = psump.tile([P, P], F32, tag="tp", name="pst2")
                        nc.tensor.transpose(
                            out=pst2[:], in_=x_sb[name, ot][:, g * P:(g + 1) * P],
                            identity=ident[:])
                        nc.vector.tensor_tensor(
                            out=xT[:, ot * P:(ot + 1) * P], in0=pst2[:],
                            in1=dcol[:].to_broadcast([P, P]), op=ALU.mult)
                    gout = psump.tile([P, O], F32, tag="gout", name="gout")
                    nc.tensor.matmul(out=gout[:], lhsT=Bm[:], rhs=xT[:],
                                     start=True, stop=True)
                    res = smallp.tile([P, O], F32, tag=f"res_{name}_{g}",
                                      name="res")
                    nc.vector.tensor_tensor(out=res[:], in0=gout[:],
                                            in1=dcol[:].to_broadcast([P, O]),
                                            op=ALU.mult)
                    nc.sync.dma_start(out=outd[8 * g:8 * (g + 1), :], in_=res[:])

    nc.compile()
    return nc


def _prep(inputs):
    fp = np.ascontiguousarray(inputs["fpam_output"], dtype=np.float32)
    N = fp.shape[0]
    fpamT = np.ascontiguousarray(
        fp.reshape(N, C, HW).transpose(0, 2, 1))          # [N, HW, C]

    def bn_fold(g, b, m, v):
        a = (g / np.sqrt(v + 1e-5)).astype(np.float32)
        bb = (b - m * a).astype(np.float32)
        return a, bb

    # packed weights: w_pk[set, p, cb*256+o] = w.T[cb*128+p, o]
    w_pk = np.empty((2, P, 8 * O), dtype=np.float32)
    for si, wname in enumerate(("w_max", "w_med")):
        wT = inputs[wname].T.astype(np.float32)           # [C, O]
        w_pk[si] = wT.reshape(8, P, O).transpose(1, 0, 2).reshape(P, 8 * O)

    # packed bn: bn_pk[p, si*4 + ot*2 + {a,b}]
    bn_pk = np.zeros((P, 8), dtype=np.float32)
    for si, nm in enumerate(("max", "med")):
        a, bb = bn_fold(inputs[f"g_{nm}"], inputs[f"b_{nm}"],
                        inputs[f"m_{nm}"], inputs[f"v_{nm}"])
        for ot in range(2):
            bn_pk[:, si * 4 + ot * 2 + 0] = a[ot * P:(ot + 1) * P]
            bn_pk[:, si * 4 + ot * 2 + 1] = bb[ot * P:(ot + 1) * P]

    j = np.arange(P)[:, None] + 128 * np.arange(4)[None, :]
    offs = ((j // (2 * K)) * HW).astype(np.uint32)        # [128, 4]
    ident = np.eye(P, dtype=np.float32)

    in_maps = []
    for c in range(NCORES):
        in_maps.append({
            "fpamT": fpamT[c * NS:(c + 1) * NS],
            "w_pk": w_pk, "bn_pk": bn_pk,
            "offs": offs, "ident": ident,
        })
    return in_maps


def kernel(**inputs):
    if "nc" not in _CACHE:
        _CACHE["nc"] = _build_bass()
    nc = _CACHE["nc"]
    in_maps = _prep(inputs)
    res = run_bass_kernel_spmd(nc, in_maps, core_ids=list(range(NCORES)))
    outs = res.results
    max_graph = np.concatenate([o["out_max"] for o in outs], axis=0)
    med_graph = np.concatenate([o["out_med"] for o in outs], axis=0)
    rows = np.concatenate([o["out_rows"] for o in outs], axis=0).astype(np.int32)
    cols = np.concatenate([o["out_cols"] for o in outs], axis=0).astype(np.int32)
    return (max_graph, med_graph, rows, cols)
